# revision 18
# baseline (speedup 1.0000x reference)
"""Trainium2 Bass kernel for nn_AblationAttention (sparse_attention).

Sharding: head-parallel attention (4 heads/core; batch b = core//4) with a
row-parallel output projection (each core computes a full-shape partial that
the host sums per batch).  One tiny AllGather(max) of [vmax, -vmin] per
4-core batch group provides the global per-batch min/max for v scaling.

Algebraic restructure vs the straightforward lowering:
  v_scaled = a*v + c with a = 1/(max-min+eps), c = -min*a.  Then
  gene_fitness rows sum to 1, so org = a*(v@gf) + c and softmax drops the
  +c.  gf = p/P with p = 1/(u + r), u = causal running mean of RAW v,
  r = -min + 0.5/a.  The exp scale a is folded into the reciprocal-sum
  matmul operand (rdb = asum*rd), so exp runs with unit scale and no
  per-partition scale AP.

Scheduling (v2): stat->AllGather chain prioritized and split across engines
so the collective fires right after the v projection; scan/transposes fill
the collective window; no DRAM bounce broadcasts (selpair matmul +
gpsimd.partition_broadcast instead); org->exp->av software pipelined so the
PE never waits on ScalarE; output projection spread across the last head.

Self-contained: hardcodes B=2, S=2048, E=1024, H=16, D=64, 8 cores.
"""
import numpy as np

import concourse.bass as bass
import concourse.mybir as mybir
import concourse.tile as tile
from concourse import bacc
from concourse.masks import make_identity
from concourse.bass_utils import run_bass_kernel_spmd

B, S, E, H, D = 2, 2048, 1024, 16, 64
NCORES = 8
HPC = H // 4                  # 4 heads per core
NJ = S // 128                 # 16 j-blocks
NT = S // 512                 # 4 i-tiles of 512
GROUPS = [[0, 1, 2, 3], [4, 5, 6, 7]]
POW = 1024                    # po psum tile width (2 banks)

F32 = mybir.dt.float32
BF = mybir.dt.bfloat16
AL = mybir.AluOpType
AX = mybir.AxisListType
AF = mybir.ActivationFunctionType


def _bcast_ap(src_ap, nrows):
    """Partition-broadcast access pattern (stride-0 partition dim)."""
    return bass.AP(tensor=src_ap.tensor, offset=src_ap.offset,
                   ap=[[0, nrows]] + list(src_ap.ap[1:]))


def build():
    nc = bacc.Bacc("TRN2", target_bir_lowering=False, debug=False,
                   num_devices=NCORES)
    # ---- I/O ----
    xT = nc.dram_tensor("xT", [E, S], BF, kind="ExternalInput")
    wvT = nc.dram_tensor("wvT", [E + 1, HPC * D], BF, kind="ExternalInput")
    wor = nc.dram_tensor("wor", [HPC * D, E], BF, kind="ExternalInput")
    rcnt = nc.dram_tensor("rcnt", [1, S], F32, kind="ExternalInput")
    trineg = nc.dram_tensor("trineg", [128, 128], BF, kind="ExternalInput")
    selpair = nc.dram_tensor("selpair", [128, 128], BF, kind="ExternalInput")
    out_part = nc.dram_tensor("out_part", [S, E], F32, kind="ExternalOutput")
    # collective bounce buffers
    ar_in = nc.dram_tensor("ar_in", [1, 2], F32)
    ar_out = nc.dram_tensor("ar_out", [1, 8], F32)

    with tile.TileContext(nc) as tc:
        with tc.tile_pool(name="const", bufs=1) as constp, \
             tc.tile_pool(name="persist", bufs=1) as persist, \
             tc.tile_pool(name="mid", bufs=1) as midp:
            identf = constp.tile([128, 128], F32)
            make_identity(nc, identf)
            rcnt_bc = constp.tile([128, S], F32)
            nc.sync.dma_start(out=rcnt_bc, in_=_bcast_ap(rcnt.ap(), 128))
            trineg_sb = constp.tile([128, 128], BF)
            nc.sync.dma_start(out=trineg_sb, in_=trineg.ap())
            identb = constp.tile([128, 128], BF)
            nc.vector.tensor_copy(out=identb, in_=identf)
            selpair_sb = constp.tile([128, 128], BF)
            nc.sync.dma_start(out=selpair_sb, in_=selpair.ap())
            mmg = constp.tile([128, 2], F32)        # [gmax, -gmin] broadcast
            asum_sb = constp.tile([128, 1], F32)
            r_sb = constp.tile([128, 1], F32)
            warm = constp.tile([1, 1], F32)
            nc.vector.memset(warm, 0.0)
            nc.scalar.activation(out=warm, in_=warm, func=AF.Exp)
            z2 = constp.tile([128, 1], F32)         # scan zero col (dep-gated)
            nc.vector.memset(z2, 0.0)

            # persistent activation tensors
            vTb = persist.tile([128, 2, S], BF)           # raw v^T, bf16
            v_s = [persist.tile([128, HPC, 66], BF, name=f"v_s{J}")
                   for J in range(NJ)]
            gfz = [persist.tile([128, 2, S], BF, name=f"gfz{p}")
                   for p in range(2)]
            ctxT = persist.tile([128, 2, S], BF)          # context^T
            wor_sb = persist.tile([128, 2, E], BF)
            for kt in range(2):
                nc.sync.dma_start(out=wor_sb[:, kt, :],
                                  in_=wor.ap()[128 * kt:128 * (kt + 1), :])
            rd = [persist.tile([128, S], F32, name=f"rd{p}") for p in range(2)]
            # zero the dead head-halves of gfz once
            for p in range(2):
                nc.vector.memset(gfz[p][64:128, 0, :], 0.0)
                nc.vector.memset(gfz[p][0:64, 1, :], 0.0)
            for J in range(NJ):
                nc.gpsimd.memset(v_s[J][:, :, 0:1], 1.0)

            u = midp.tile([128, 2, S], F32)
            smx = midp.tile([128, 8], F32)
            smn = midp.tile([128, 8], F32)
            stat = midp.tile([128, 2], F32)
            st2 = midp.tile([2, 1], F32)

            # ---------- Phase A: v projection (bf16 x @ wv^T) ----------
            with tc.tile_pool(name="xw", bufs=1) as xw, \
                 tc.tile_pool(name="psA", bufs=1, space="PSUM") as psA:
                wvT_sb = xw.tile([128, 8, HPC * D], BF)
                xks = []
                for k in range(8):
                    # interleave weight + activation chunk loads so the
                    # k-loop below starts as soon as chunk 0 lands
                    nc.sync.dma_start(
                        out=wvT_sb[:, k, :],
                        in_=wvT.ap()[128 * k:128 * (k + 1), :])
                    xk = xw.tile([128, S], BF, name=f"xk{k}")
                    nc.sync.dma_start(out=xk,
                                      in_=xT.ap()[128 * k:128 * (k + 1), :])
                    xks.append(xk)
                wvb9 = xw.tile([1, HPC * D], BF)
                nc.sync.dma_start(out=wvb9, in_=wvT.ap()[E:E + 1, :])
                ones1 = xw.tile([1, S], BF)
                nc.vector.memset(ones1, 1.0)
                pv = [psA.tile([128, 512], F32, name=f"pv{i}")
                      for i in range(8)]
                # k-outer so matmuls start after the first 128-chan chunk
                # of x lands instead of after all 4MB
                for m in range(2):
                    for k in range(8):
                        for t in range(NT):
                            nc.tensor.matmul(
                                pv[m * NT + t],
                                wvT_sb[:, k, 128 * m:128 * (m + 1)],
                                xks[k][:, 512 * t:512 * (t + 1)],
                                start=(k == 0), stop=False)
                    for t in range(NT):
                        i = m * NT + t
                        nc.tensor.matmul(
                            pv[i], wvb9[:, 128 * m:128 * (m + 1)],
                            ones1[:, 512 * t:512 * (t + 1)],
                            start=False, stop=True)
                        nc.scalar.activation(
                            out=vTb[:, m, 512 * t:512 * (t + 1)],
                            in_=pv[i], func=AF.Copy)
                        nc.vector.tensor_reduce(
                            out=smx[:, i:i + 1], in_=pv[i],
                            op=AL.max, axis=AX.X)
                        nc.vector.tensor_reduce(
                            out=smn[:, i:i + 1], in_=pv[i],
                            op=AL.min, axis=AX.X)
                # stats finalize; the scans are gated behind st2 (via z2)
                # so the greedy scheduler can't delay the collective
                nc.vector.tensor_reduce(out=stat[:, 0:1], in_=smx,
                                        op=AL.max, axis=AX.X)
                nc.vector.tensor_reduce(out=stat[:, 1:2], in_=smn,
                                        op=AL.min, axis=AX.X)
                nc.vector.tensor_scalar_mul(out=stat[:, 1:2],
                                            in0=stat[:, 1:2], scalar1=-1.0)
                pstat = psA.tile([2, 128], F32, name="pv0")
                nc.tensor.transpose(pstat, stat, identf)
                nc.vector.tensor_reduce(out=st2, in_=pstat, op=AL.max,
                                        axis=AX.X)
                nc.sync.dma_start(out=ar_in.ap(), in_=st2)
                nc.gpsimd.collective_compute(
                    "AllGather", AL.bypass, replica_groups=GROUPS,
                    ins=[ar_in.ap()], outs=[ar_out.ap()])

            # ---------- AR-independent work (fills collective latency) ----
            # z2[0:2] = 0*st2 gates the scans behind the whole stat chain so
            # the greedy scheduler can't slot a 5us scan ahead of it.
            nc.vector.tensor_scalar_mul(out=z2[0:2, :], in0=st2, scalar1=0.0)
            # u = causal running mean of raw v (scan then * 1/(i+1))
            for p in range(2):
                zd = bass.AP(tensor=z2.tensor, offset=z2.offset,
                             ap=[list(z2.ap[0]), [0, S]])
                nc.vector.tensor_tensor_scan(
                    out=u[:, p, :], data0=vTb[:, p, :], data1=zd,
                    initial=0.0, op0=AL.add, op1=AL.add)
                nc.vector.tensor_tensor(out=u[:, p, :], in0=u[:, p, :],
                                        in1=rcnt_bc, op=AL.mult)
            # transposes: v_s[J] = v block [j, (h, d)]
            with tc.tile_pool(name="psT", bufs=2, space="PSUM") as psT:
                for J in range(NJ):
                    ptr = psT.tile([128, 256], BF, name="ptr")
                    for m in range(2):
                        nc.tensor.transpose(
                            ptr[:, 128 * m:128 * (m + 1)],
                            vTb[:, m, 128 * J:128 * (J + 1)], identb)
                    nc.scalar.activation(
                        out=v_s[J][:, :, 1:D + 1],
                        in_=ptr.rearrange("p (h d) -> p h d", h=HPC),
                        func=AF.Copy)

            # ---------- AR result -> asum, r ----------
            mmg8 = constp.tile([128, 8], F32)
            nc.sync.dma_start(out=mmg8, in_=_bcast_ap(ar_out.ap(), 128))
            view = bass.AP(tensor=mmg8.tensor, offset=mmg8.offset,
                           ap=[list(mmg8.ap[0]), [1, 2], [2, 4]])
            nc.vector.tensor_reduce(out=mmg, in_=view, op=AL.max, axis=AX.X)
            nc.vector.tensor_tensor(out=asum_sb, in0=mmg[:, 0:1],
                                    in1=mmg[:, 1:2], op=AL.add)
            nc.vector.tensor_scalar_add(out=asum_sb, in0=asum_sb,
                                        scalar1=1e-8)
            # r = 0.5*asum + (-min)
            nc.vector.tensor_scalar(out=r_sb, in0=asum_sb, scalar1=0.5,
                                    scalar2=mmg[:, 1:2], op0=AL.mult,
                                    op1=AL.add)

            # ---------- Phases C + D + F ----------
            with tc.tile_pool(name="cw", bufs=3) as cw, \
                 tc.tile_pool(name="etp", bufs=5) as etp, \
                 tc.tile_pool(name="zp", bufs=2) as zp, \
                 tc.tile_pool(name="psO", bufs=2, space="PSUM") as psO, \
                 tc.tile_pool(name="psPav", bufs=1, space="PSUM") as psPav:

                def phase_c(p):
                    # chunk-granular so the first org can start after the
                    # first half of gfz[p] is ready
                    for cchunk in range(4):
                        sl = slice(512 * cchunk, 512 * (cchunk + 1))
                        den = cw.tile([128, 512], F32, name="den")
                        nc.vector.tensor_scalar_add(out=den, in0=u[:, p, sl],
                                                    scalar1=r_sb)
                        nc.vector.reciprocal_approx_fast(out=rd[p][:, sl],
                                                         in_=den)
                        # rdb = asum * rd folds the exp scale a=1/asum into
                        # gfz via rs = 1/(asum*si)
                        rdb = cw.tile([128, 512], BF, name="rdb")
                        nc.vector.tensor_scalar_mul(out=rdb,
                                                    in0=rd[p][:, sl],
                                                    scalar1=asum_sb)
                        si = psO.tile([128, 512], F32, name="po")
                        nc.tensor.matmul(si, selpair_sb, rdb,
                                         start=True, stop=True)
                        rs = cw.tile([128, 512], F32, name="rs")
                        nc.vector.reciprocal_approx_fast(out=rs, in_=si)
                        nc.vector.tensor_tensor(
                            out=gfz[p][0:64, 0, sl], in0=rd[p][0:64, sl],
                            in1=rs[0:64, :], op=AL.mult)
                        nc.gpsimd.tensor_tensor(
                            out=gfz[p][64:128, 1, sl], in0=rd[p][64:128, sl],
                            in1=rs[64:128, :], op=AL.mult)

                def phase_f(mts):
                    for mt in mts:
                        poo = psO.tile([128, 1024], F32, name="po")
                        for nt in range(2):
                            for kt in range(2):
                                nc.tensor.matmul(
                                    poo[:, 512 * nt:512 * (nt + 1)],
                                    ctxT[:, kt, 128 * mt:128 * (mt + 1)],
                                    wor_sb[:, kt, 512 * nt:512 * (nt + 1)],
                                    start=(kt == 0), stop=(kt == 1))
                        osb = cw.tile([128, 1024], F32, name="osb")
                        if mt % 2 == 0:
                            nc.vector.tensor_copy(out=osb, in_=poo)
                        else:
                            nc.scalar.activation(out=osb, in_=poo,
                                                 func=AF.Copy)
                        nc.sync.dma_start(
                            out=out_part.ap()[128 * mt:128 * (mt + 1), :],
                            in_=osb)

                def epilogue(p, s, T, pav):
                    # pav row 0 = Z, rows 1:65 = ctx^T for head (p, s)
                    cols = slice(512 * T, 512 * (T + 1))
                    rz = zp.tile([1, 512], F32, name="rz")
                    nc.vector.reciprocal_approx_fast(out=rz,
                                                     in_=pav[0:1, :])
                    rzb = zp.tile([65, 512], F32, name="rzb")
                    nc.gpsimd.partition_broadcast(rzb, rz)
                    ctmp = zp.tile([65, 512], BF, name="ctmp")
                    nc.vector.tensor_tensor(
                        out=ctmp[0:64, :], in0=pav[0:64, :],
                        in1=rzb[0:64, :], op=AL.mult)
                    nc.vector.tensor_tensor(
                        out=ctmp[64:65, :], in0=pav[64:65, :],
                        in1=rzb[64:65, :], op=AL.mult)
                    nc.sync.dma_start(out=ctxT[64 * s:64 * (s + 1), p, cols],
                                      in_=ctmp[1:65, :])

                # ---- main attention loop, software-pipelined ----
                phase_c(0)
                for lh in range(4):
                    p, s = lh // 2, lh % 2
                    pavs = [psPav.tile([65, 512], F32, name=f"pav{T}")
                            for T in range(NT)]

                    # build the chunk work-list for this head
                    units = []
                    for J in range(NJ):
                        Tj = J // 4
                        c0g = 128 * J
                        m1 = min(S, 512 * Tj + 1024)
                        chunks = [(c0g, m1)] + ([(m1, S)] if m1 < S else [])
                        for ci, (cs, ce) in enumerate(chunks):
                            units.append((J, cs, ce, ci == len(chunks) - 1))

                    def emit_org_exp(unit):
                        J, cs, ce, _ = unit
                        Tj = J // 4
                        c0g = 128 * J
                        base = 512 * Tj if cs == c0g else cs
                        po = psO.tile([128, POW], F32, name="po")
                        diag = cs == c0g
                        for T in range(Tj, NT):
                            tcs = max(cs, 512 * T)
                            tce = min(ce, 512 * (T + 1))
                            if tcs >= tce:
                                continue
                            if diag and T == Tj:
                                # causal mask folded in: -100 above the
                                # diagonal makes exp() flush those to zero.
                                # Mask first (start), then accumulate the
                                # full-range org on top (stop).
                                nc.tensor.matmul(
                                    po[:, c0g - base:c0g - base + 128],
                                    identb, trineg_sb,
                                    start=True, stop=False)
                                nc.tensor.matmul(
                                    po[:, tcs - base:tce - base],
                                    vTb[:, p, 128 * J:128 * (J + 1)],
                                    gfz[p][:, s, tcs:tce],
                                    start=False, stop=True,
                                    skip_group_check=True)
                                continue
                            nc.tensor.matmul(
                                po[:, tcs - base:tce - base],
                                vTb[:, p, 128 * J:128 * (J + 1)],
                                gfz[p][:, s, tcs:tce],
                                start=True, stop=True)
                        w = ce - cs
                        et = etp.tile([128, POW], BF, name="et")
                        nc.scalar.activation(
                            out=et[:, 0:w], in_=po[:, cs - base:ce - base],
                            func=AF.Exp)
                        return (J, cs, ce, et)

                    def emit_av(st):
                        J, cs, ce, et = st
                        for T in range(J // 4, NT):
                            tcs = max(cs, 512 * T)
                            tce = min(ce, 512 * (T + 1))
                            if tcs >= tce:
                                continue
                            nc.tensor.matmul(
                                pavs[T][:, tcs - 512 * T:tce - 512 * T],
                                v_s[J][:, lh, 0:65],
                                et[:, tcs - cs:tce - cs],
                                start=(J == 0), stop=(J == 4 * T + 3),
                                skip_group_check=True)

                    pending = None
                    for ui, unit in enumerate(units):
                        st = emit_org_exp(unit)
                        if pending is not None:
                            emit_av(pending)
                            J, cs, ce, _ = pending
                            if ce == S and J % 4 == 3:
                                T = J // 4
                                epilogue(p, s, T, pavs[T])
                                if lh == 3:
                                    phase_f(range(4 * T, 4 * T + 4))
                        pending = st
                        if lh == 0 and unit[0] == 2 and unit[3]:
                            phase_c(1)
                    emit_av(pending)
                    T = NT - 1
                    epilogue(p, s, T, pavs[T])
                    if lh == 3:
                        phase_f(range(4 * T, 4 * T + 4))

    nc.compile()
    return nc


BF_NP = mybir.dt.np(BF)


def make_host_inputs(x, wv_w, wv_b, wo_w, wo_b):
    """Per-core input dicts (host-side sharding)."""
    rcnt = (1.0 / (np.arange(S, dtype=np.float64) + 1.0)).astype(np.float32)
    jj = np.arange(128)[:, None]
    ii = np.arange(128)[None, :]
    trineg = np.where(jj <= ii, 0.0, -100.0).astype(BF_NP)
    selpair = ((jj < 64) == (ii < 64)).astype(BF_NP)
    woT = np.ascontiguousarray(wo_w.T)
    in_maps = []
    for c in range(NCORES):
        b, q = c // 4, c % 4
        csl = slice(HPC * D * q, HPC * D * (q + 1))
        in_maps.append({
            "xT": np.ascontiguousarray(x[b].T).astype(BF_NP),
            "wvT": np.ascontiguousarray(
                np.concatenate([wv_w[csl, :].T, wv_b[None, csl]],
                               axis=0)).astype(BF_NP),
            "wor": np.ascontiguousarray(woT[csl, :]).astype(BF_NP),
            "rcnt": rcnt.reshape(1, S),
            "trineg": trineg,
            "selpair": selpair,
        })
    return in_maps


_NC_CACHE = {}


def _get_nc():
    if "nc" not in _NC_CACHE:
        _NC_CACHE["nc"] = build()
    return _NC_CACHE["nc"]


def _assemble(results, wo_b):
    out = np.zeros((B, S, E), np.float32)
    for c in range(NCORES):
        out[c // 4] += results[c]["out_part"]
    out += wo_b[None, None, :]
    return out


def kernel(x, wv_w, wv_b, wo_w, wo_b):
    x = np.asarray(x, np.float32)
    wo_b = np.asarray(wo_b, np.float32)
    in_maps = make_host_inputs(
        x, np.asarray(wv_w, np.float32), np.asarray(wv_b, np.float32),
        np.asarray(wo_w, np.float32), wo_b)
    nc = _get_nc()
    res = run_bass_kernel_spmd(nc, in_maps, core_ids=list(range(NCORES)))
    return _assemble(res.results, wo_b)


def run_traced(x, wv_w, wv_b, wo_w, wo_b, trace_cores=None):
    wo_b = np.asarray(wo_b, np.float32)
    in_maps = make_host_inputs(
        np.asarray(x, np.float32), np.asarray(wv_w, np.float32),
        np.asarray(wv_b, np.float32), np.asarray(wo_w, np.float32), wo_b)
    nc = _get_nc()
    res = run_bass_kernel_spmd(nc, in_maps, core_ids=list(range(NCORES)),
                               trace=True, trace_cores=trace_cores)
    return _assemble(res.results, wo_b), res
